# revision 1
# baseline (speedup 1.0000x reference)
"""Trainium2 Bass kernel for CapsNet dynamic routing (nn_Model_16492674417055).

Reference computation:
    u_hat[b,i,j,c,p] = sum_q w[j,c,p,q] x[b,i,c,q]
    3 routing iterations of: c = softmax_j(b); s = sum_i c*u_hat;
    v = squash(s); a = <u_hat, v>; b += a. Output v of last iteration.

Key algebraic factorization (exact in real arithmetic): u_hat never needs to
be materialized (it is 1 GiB).  With xc[b,j,c,:] = sum_i c[b,i,j,c] x[b,i,c,:]:
    s  = W @ xc
    a  = <x_i, W^T v>  and  W^T v = kappa * (W^T W) xc = kappa * G xc,
where kappa is the squash scale, computable from |s|^2 = <xc, G xc>.
So iterations 1..2 need only G = W^T W (host-precomputed), and the final
iteration needs one true W application for the output direction.

Sharding: data-parallel over batch B=16 across 8 cores (2 batches/core);
G / wT are replicated (loaded per core); routing state stays core-local.
All compute in fp32 (bf16 breaks the sharp routing softmax: measured 1e-2+
relative error vs 6e-6 for fp32).

Memory plan (per core, per partition): G chunk tiles 64K + xk 32K + xt 32K
+ logits/softmax scratch ~25K, all fp32. wT is streamed at t=1 into the G
tiles right after each chunk's last G-read, so only one 8 MiB w-derived
buffer is resident. DMA is split across both HWDGE rings (nc.sync and
nc.scalar) in 1-4 MiB chunks so compute starts early.
"""

import numpy as np

import concourse.bass as bass
import concourse.tile as tile
from concourse import bacc
from concourse import mybir
from concourse.bass import MemorySpace
from concourse.bass_utils import run_bass_kernel_spmd
from concourse.masks import make_identity

F32 = mybir.dt.float32
AXX = mybir.AxisListType.X
AF = mybir.ActivationFunctionType

N_CORES = 8
B, N_PRE, N_DIGIT, CH, D = 16, 1024, 32, 4, 128
BL = B // N_CORES          # batches per core (2)
NCHUNK = N_PRE // 128      # i-chunks (8)
NJC = N_DIGIT * CH         # 128 (j,c) pairs
GCH = 16                   # (j,c) tiles per G DMA chunk
NGC = NJC // GCH           # 8 G chunks
EPS = 1e-7
N_ITERS = 3


class _Bacc(bacc.Bacc):
    """Bacc whose ACT-table chooser only sees natural_log_exp_and_others, so
    alternating Exp (softmax) / Ln+Exp (squash sqrt) stay on ONE table set
    (one LoadActFuncSet instead of one per switch)."""

    def insert_act_table_loads(self):
        from concourse.hw_specs import get_activation_tables

        has_activation = any(
            isinstance(i, mybir.InstActivation)
            for b in self.main_func.blocks
            for i in b.instructions
        )
        if not has_activation:
            return
        tables = [
            (n, fns if n == "natural_log_exp_and_others" else set())
            for n, fns in get_activation_tables(self.m.arch).items()
        ]
        bacc._bass_rust.insert_act_table_loads(self, tables)


def build_nc(bench_reps: int = 0, bench_mode: str = "full") -> bass.Bass:
    """bench_reps>0 wraps the whole kernel body (input DMAs included) in a
    For_i loop of that many reps inside one NEFF, for wall-clock timing that
    amortizes the multi-ms axon dispatch floor. In bench mode the t=1 wT
    refill targets a dummy tile (same DMA traffic) so the G tiles stay valid
    for the next rep; t=2 then multiplies by G instead of W^T, which changes
    values but not the instruction stream being timed."""
    nc = _Bacc()

    # Per-core DRAM inputs, host pre-laid-out so every load is a straight
    # [128, N] partition-major copy.
    xk_d = nc.declare_dram_parameter("xk", [128, BL, CH, NCHUNK, 128], F32, isOutput=False)  # [i128, b, c, k, q]
    xt_d = nc.declare_dram_parameter("xt", [128, BL, CH, NCHUNK, 128], F32, isOutput=False)  # [q, b, c, k, i128]
    g_d = nc.declare_dram_parameter("g", [128, NJC, 128], F32, isOutput=False)               # [q, (j c), q']
    wt_d = nc.declare_dram_parameter("wt", [128, NJC, 128], F32, isOutput=False)             # [q, (j c), p]
    out_d = nc.declare_dram_parameter("out", [BL, N_DIGIT, CH, D], F32, isOutput=True)

    with tile.TileContext(nc) as tc:
        with (
            tc.tile_pool(name="big", bufs=1) as big,
            tc.tile_pool(name="sm", bufs=2) as sm,
            tc.tile_pool(name="ps_xc", bufs=2, space=MemorySpace.PSUM) as ps_xc,
            tc.tile_pool(name="ps_gx", bufs=2, space=MemorySpace.PSUM) as ps_gx,
            tc.tile_pool(name="ps_a", bufs=2, space=MemorySpace.PSUM) as ps_a,
            tc.tile_pool(name="ps_sq", bufs=1, space=MemorySpace.PSUM) as ps_sq,
            tc.tile_pool(name="ps_kb", bufs=1, space=MemorySpace.PSUM) as ps_kb,
        ):
            # ---- static tiles ----
            xk = big.tile([128, BL, CH, NCHUNK, 128], F32, tag="xk")
            xt = big.tile([128, BL, CH, NCHUNK, 128], F32, tag="xt")
            gch = [
                big.tile([128, GCH, 128], F32, tag=f"g{gc}", name=f"g{gc}")
                for gc in range(NGC)
            ]  # noqa
            wt_dummy = None
            if bench_reps:
                wt_dummy = big.tile([128, GCH, 128], F32, tag="wt_dummy", name="wt_dummy")

            def wtile(jc):
                return gch[jc // GCH][:, jc % GCH, :]

            c_unif = big.tile([128, N_DIGIT], F32, tag="c_unif")
            nc.vector.memset(c_unif, 1.0 / N_DIGIT)
            ones_col = big.tile([128, 1], F32, tag="ones_col")
            nc.vector.memset(ones_col, 1.0)
            ones_row = big.tile([1, 128], F32, tag="ones_row")
            nc.vector.memset(ones_row, 1.0)
            ident = big.tile([128, 128], F32, tag="ident")
            make_identity(nc, ident[:])
            eps_t = big.tile([1, 1], F32, tag="eps_t")
            nc.vector.memset(eps_t, EPS)

            # routing logits per local batch: [i%128, k, c, j]
            bl_t = [big.tile([128, NCHUNK, CH, N_DIGIT], F32, tag=f"bl{b}", name=f"bl{b}") for b in range(BL)]

            def trace_loads():
              # per-b splits: XC(b=0) starts after 2 MiB instead of 4 MiB
              for b in range(BL):
                nc.sync.dma_start(out=xk[:, b], in_=xk_d[:, b])
              for b in range(BL):
                nc.sync.dma_start(out=xt[:, b], in_=xt_d[:, b])
              for gc in range(NGC):
                nc.scalar.dma_start(
                    out=gch[gc][:], in_=g_d[:, gc * GCH : (gc + 1) * GCH, :]
                )

            def trace_body(loads=True, compute=True):
              # ---- input loads (two HWDGE rings: sync for x, scalar for G) ----
              if loads:
                trace_loads()
              if not compute:
                return
              for t in range(N_ITERS):
                  last = t == N_ITERS - 1

                  # ---- softmax over j (t=0: uniform, skip) ----
                  cbt = []
                  if t > 0:
                      for b in range(BL):
                          # b=0 elementwise on DVE, b=1 on GpSimd (parallel)
                          ve = nc.vector if b == 0 else nc.gpsimd
                          blv = bl_t[b][:]
                          mx = sm.tile([128, NCHUNK, CH], F32, tag=f"mx{b}")
                          nc.vector.reduce_max(out=mx[:], in_=blv, axis=AXX, negate=True)
                          eb = sm.tile([128, NCHUNK, CH, N_DIGIT], F32, tag=f"e{b}")
                          ve.tensor_add(eb[:], blv, mx[:].to_broadcast(eb.shape))
                          nc.scalar.activation(eb[:], eb[:], AF.Exp)
                          sb = sm.tile([128, NCHUNK, CH], F32, tag=f"sum{b}")
                          nc.vector.reduce_sum(out=sb[:], in_=eb[:], axis=AXX)
                          nc.vector.reciprocal(sb[:], sb[:])
                          cb = sm.tile([128, NCHUNK, CH, N_DIGIT], F32, tag=f"C{b}")
                          ve.tensor_mul(cb[:], eb[:], sb[:].to_broadcast(eb.shape))
                          cbt.append(cb)

                  # ---- XC: xcT[q, (j,b)] per c ----
                  xc_sb = [sm.tile([128, N_DIGIT, BL], F32, tag=f"xc{c}", name=f"xc{c}", bufs=3) for c in range(CH)]
                  for c in range(CH):
                      for b in range(BL):
                          xc_ps = ps_xc.tile([128, N_DIGIT], F32, tag="xc_ps")
                          for k in range(NCHUNK):
                              rhs = cbt[b][:, k, c, :] if t > 0 else c_unif[:]
                              nc.tensor.matmul(
                                  xc_ps[:],
                                  lhsT=xk[:, b, c, k, :],
                                  rhs=rhs,
                                  start=(k == 0),
                                  stop=(k == NCHUNK - 1),
                              )
                          nc.vector.tensor_copy(xc_sb[c][:, :, b], xc_ps[:])

                  # ---- W-pass: gxcT = G @ xc (t<2)  /  sT = W @ xc (t=2) ----
                  gx_ps = ps_gx.tile([128, CH, N_DIGIT, BL], F32, tag="gx")
                  for jc in range(NJC):
                      c, j = divmod(jc, N_DIGIT)
                      nc.tensor.matmul(
                          gx_ps[:, c, j, :],
                          lhsT=wtile(jc),
                          rhs=xc_sb[c][:, j, :],
                          start=True,
                          stop=True,
                      )
                      if t == 1 and jc % GCH == GCH - 1:
                          # whole chunk's last G read done -> refill with wT
                          gc = jc // GCH
                          nc.scalar.dma_start(
                              out=gch[gc][:],
                              in_=wt_d[:, gc * GCH : (gc + 1) * GCH, :],
                          )

                  # ---- per-c: squash scale kappa, vt = kappa*gx ----
                  # pipelines with the W-pass: c's chain starts when its 32
                  # W matmuls are done, while the W-pass continues on c+1.
                  gx_sb = sm.tile([128, CH, N_DIGIT, BL], F32, tag="gx_sb", bufs=3)
                  xg = sm.tile([128, CH, N_DIGIT, BL], F32, tag="xg")
                  sq_ps = ps_sq.tile([1, CH, N_DIGIT * BL], F32, tag="sq")
                  kb_ps = ps_kb.tile([128, CH, N_DIGIT, BL], F32, tag="kb")
                  t1 = sm.tile([1, CH, N_DIGIT * BL], F32, tag="t1")
                  t2 = sm.tile([1, CH, N_DIGIT * BL], F32, tag="t2")
                  kap = sm.tile([1, CH, N_DIGIT * BL], F32, tag="kap")
                  vt = sm.tile([128, CH, N_DIGIT, BL], F32, tag="vt", bufs=3)
                  for c in range(CH):
                      nc.scalar.copy(out=gx_sb[:, c], in_=gx_ps[:, c])
                      # t<2: |s|^2 = <xc, G xc>;  t=2: |s|^2 = <s, s>
                      if not last:
                          nc.vector.tensor_mul(
                              xg[:, c, :, :], xc_sb[c][:, :, :], gx_sb[:, c, :, :]
                          )
                      else:
                          nc.vector.tensor_mul(xg[:, c], gx_sb[:, c], gx_sb[:, c])
                      nc.tensor.matmul(
                          sq_ps[:, c],
                          lhsT=ones_col[:],
                          rhs=xg[:, c].rearrange("p a b -> p (a b)"),
                          start=True,
                          stop=True,
                      )
                      # kappa = sq/((1+sq)*sqrt(sq+eps)); sqrt = exp(0.5*ln) so
                      # only the natural_log_exp ACT table set is used.
                      nc.scalar.activation(t1[:, c], sq_ps[:, c], AF.Ln, bias=eps_t[:])
                      nc.scalar.activation(t1[:, c], t1[:, c], AF.Exp, scale=0.5)
                      from concourse.alu_op_type import AluOpType as _AO
                      nc.vector.scalar_tensor_tensor(
                          out=t2[:, c], in0=sq_ps[:, c], scalar=1.0,
                          in1=t1[:, c], op0=_AO.add, op1=_AO.mult,
                      )
                      nc.vector.reciprocal(t2[:, c], t2[:, c])
                      nc.vector.tensor_mul(kap[:, c], sq_ps[:, c], t2[:, c])
                      nc.tensor.matmul(
                          kb_ps[:, c].rearrange("p a b -> p (a b)"),
                          lhsT=ones_row[:],
                          rhs=kap[:, c],
                          start=True,
                          stop=True,
                      )
                      # vt (t<2) or v (t=2): kappa * gx
                      nc.vector.tensor_mul(vt[:, c], gx_sb[:, c], kb_ps[:, c])

                  if not last:
                      # ---- A-pass: a[i,j] = sum_q x[i,q] vt[j,q]; b += a ----
                      for b in range(BL):
                          for k in range(NCHUNK):
                              a_ps = ps_a.tile([128, CH, N_DIGIT], F32, tag="a")
                              for c in range(CH):
                                  nc.tensor.matmul(
                                      a_ps[:, c, :],
                                      lhsT=xt[:, b, c, k, :],
                                      rhs=vt[:, c, :, b],
                                      start=True,
                                      stop=True,
                                  )
                              if t == 0:
                                  nc.vector.tensor_copy(bl_t[b][:, k], a_ps[:])
                              else:
                                  nc.vector.tensor_add(bl_t[b][:, k], bl_t[b][:, k], a_ps[:])
                  else:
                      # ---- output: transpose v [p, (c,j,b)] -> [(c,j,b), p], DMA ----
                      vflat = vt[:].rearrange("p a b c -> p (a b c)")
                      out_ap = out_d[:].rearrange("b j c p -> c j b p")  # [4,32,2,128]
                      for half in range(2):
                          tr_ps = ps_a.tile([128, 128], F32, tag="a")
                          nc.tensor.transpose(
                              tr_ps[:], vflat[:, half * 128 : (half + 1) * 128], ident[:]
                          )
                          ob = sm.tile([128, 128], F32, tag=f"ob{half}")
                          nc.vector.tensor_copy(ob[:], tr_ps[:])
                          for cl in range(2):
                              nc.sync.dma_start(
                                  out=out_ap[half * 2 + cl],
                                  in_=ob[cl * 64 : (cl + 1) * 64, :],
                              )

            if bench_reps:
                if bench_mode == "nodma":
                    trace_loads()
                with tc.For_i(0, bench_reps, 1):
                    trace_body(loads=(bench_mode != "nodma"),
                               compute=(bench_mode != "dmaonly"))
            else:
                trace_body()
    return nc


def _host_prep(x: np.ndarray, w: np.ndarray):
    """Host-side layout prep shared by all cores (w-derived) and per-core (x)."""
    x = np.ascontiguousarray(x, dtype=np.float32)
    w = np.ascontiguousarray(w, dtype=np.float32)
    # G[j,c,q,r] = sum_p w[j,c,p,q] w[j,c,p,r]
    wf = np.ascontiguousarray(w.transpose(1, 0, 2, 3)).reshape(NJC, D, D)  # jc = c*32+j
    G = np.matmul(wf.transpose(0, 2, 1), wf)                 # [jc, q, r]
    g_h = np.ascontiguousarray(G.transpose(1, 0, 2))         # [q, jc, r]
    wt_h = np.ascontiguousarray(wf.transpose(2, 0, 1))       # [q, jc, p]
    # x[b,i,c,q] with i = k*128 + r  ->  xk [r, b, c, k, q], xt [q, b, c, k, r]
    xr = x.reshape(B, NCHUNK, 128, CH, D)
    xk_h = np.ascontiguousarray(xr.transpose(2, 0, 3, 1, 4))  # [r, b, c, k, q]
    xt_h = np.ascontiguousarray(xr.transpose(4, 0, 3, 1, 2))  # [q, b, c, k, r]
    return xk_h, xt_h, g_h, wt_h


def _run(x: np.ndarray, w: np.ndarray, **spmd_kwargs):
    xk_h, xt_h, g_h, wt_h = _host_prep(x, w)
    in_maps = []
    for core in range(N_CORES):
        in_maps.append(
            {
                "xk": xk_h[:, core * BL : (core + 1) * BL],
                "xt": xt_h[:, core * BL : (core + 1) * BL],
                "g": g_h,
                "wt": wt_h,
            }
        )
    nc = build_nc()
    nc.finalize()
    res = run_bass_kernel_spmd(nc, in_maps, list(range(N_CORES)), **spmd_kwargs)
    out = np.concatenate([res.results[c]["out"] for c in range(N_CORES)], axis=0)
    return out.astype(np.float32), res


def kernel(x: np.ndarray, w: np.ndarray) -> np.ndarray:
    out, _ = _run(x, w)
    return out



# revision 7
# speedup vs baseline: 4.8814x; 4.8814x over previous
"""Trainium2 Bass kernel for CapsNet dynamic routing (nn_Model_16492674417055).

Reference computation:
    u_hat[b,i,j,c,p] = sum_q w[j,c,p,q] x[b,i,c,q]
    3 routing iterations of: c = softmax_j(b); s = sum_i c*u_hat;
    v = squash(s); a = <u_hat, v>; b += a. Output v of last iteration.

Key algebraic factorization (exact in real arithmetic): u_hat never needs to
be materialized (it is 1 GiB).  With xc[b,j,c,:] = sum_i c[b,i,j,c] x[b,i,c,:]:
    s  = W @ xc
    a  = <x_i, W^T v>  and  W^T v = kappa * (W^T W) xc = kappa * G xc,
where kappa is the squash scale, computable from |s|^2 = <xc, G xc>.
So iterations 1..2 need only G = W^T W (host-precomputed), and the final
iteration needs one true W application for the output direction.

Precision: all matmul inputs are fp16 (x, G, wt, softmax weights c, xc, vt);
accumulation is fp32 in PSUM; logits/softmax/squash scalars are fp32.  fp16
(10 mantissa bits) keeps the sharp routing softmax accurate: measured 2.0e-3
relative error on the seed-0 inputs (vs 1.6e-2 for bf16, 3.7e-5 for fp32).
The xc*gx products reach ~6e5 > fp16 max, so xg stays fp32 (fp32 ones-matmul
for |s|^2). fp16 also quarters PE matmul cost vs fp32 and halves DMA.

Sharding: data-parallel over batch B=16 across 8 cores (2 batches/core);
G / wT are replicated (loaded per core); routing state stays core-local.
"""

import numpy as np

import concourse.bass as bass
import concourse.tile as tile
from concourse import bacc
from concourse import mybir
from concourse.alu_op_type import AluOpType as AO
from concourse.bass import MemorySpace
from concourse.bass_utils import run_bass_kernel_spmd
from concourse.masks import make_identity

F32 = mybir.dt.float32
F16 = mybir.dt.float16
AXX = mybir.AxisListType.X
AF = mybir.ActivationFunctionType

N_CORES = 8
B, N_PRE, N_DIGIT, CH, D = 16, 1024, 32, 4, 128
BL = B // N_CORES          # batches per core (2)
NCHUNK = N_PRE // 128      # i-chunks (8)
NJC = N_DIGIT * CH         # 128 (j,c) pairs
EPS = 1e-7
N_ITERS = 3


class _Bacc(bacc.Bacc):
    """Bacc whose ACT-table chooser only sees natural_log_exp_and_others, so
    alternating Exp (softmax) / Ln+Exp (squash sqrt) stay on ONE table set
    (one LoadActFuncSet instead of one per switch)."""

    def insert_act_table_loads(self):
        from concourse.hw_specs import get_activation_tables

        has_activation = any(
            isinstance(i, mybir.InstActivation)
            for b in self.main_func.blocks
            for i in b.instructions
        )
        if not has_activation:
            return
        tables = [
            (n, fns if n == "natural_log_exp_and_others" else set())
            for n, fns in get_activation_tables(self.m.arch).items()
        ]
        bacc._bass_rust.insert_act_table_loads(self, tables)


def build_nc(bench_reps: int = 0, bench_mode: str = "full") -> bass.Bass:
    """bench_reps>0 wraps the whole kernel body (input DMAs included) in a
    For_i loop of that many reps inside one NEFF, for wall-clock timing that
    amortizes the multi-ms axon dispatch floor."""
    nc = _Bacc()

    # Per-core DRAM inputs, host pre-laid-out so every load is a straight
    # [128, N] partition-major copy.  All fp16.
    xk_d = nc.declare_dram_parameter("xk", [128, BL, CH, NCHUNK, 128], F16, isOutput=False)  # [i128, b, c, k, q]
    xt_d = nc.declare_dram_parameter("xt", [128, BL, CH, NCHUNK, 128], F16, isOutput=False)  # [q, b, c, k, i128]
    g_d = nc.declare_dram_parameter("g", [128, NJC, 128], F16, isOutput=False)               # [r, (c j), q]
    wt_d = nc.declare_dram_parameter("wt", [128, NJC, 128], F16, isOutput=False)             # [q, (c j), p]
    out_d = nc.declare_dram_parameter("out", [BL, N_DIGIT, CH, D], F32, isOutput=True)

    with tile.TileContext(nc) as tc:
        with (
            tc.tile_pool(name="big", bufs=1) as big,
            tc.tile_pool(name="sm", bufs=2) as sm,
            tc.tile_pool(name="ps_xc", bufs=2, space=MemorySpace.PSUM) as ps_xc,
            tc.tile_pool(name="ps_gx", bufs=2, space=MemorySpace.PSUM) as ps_gx,
            tc.tile_pool(name="ps_a", bufs=2, space=MemorySpace.PSUM) as ps_a,
            tc.tile_pool(name="ps_sq", bufs=1, space=MemorySpace.PSUM) as ps_sq,
            tc.tile_pool(name="ps_kb", bufs=1, space=MemorySpace.PSUM) as ps_kb,
        ):
            # ---- static tiles ----
            xk = big.tile([128, BL, CH, NCHUNK, 128], F16, tag="xk")
            xt = big.tile([128, BL, CH, NCHUNK, 128], F16, tag="xt")
            g_t = big.tile([128, NJC, 128], F16, tag="g")
            wt_t = big.tile([128, NJC, 128], F16, tag="wt")

            c_unif = big.tile([128, N_DIGIT], F16, tag="c_unif")
            nc.vector.memset(c_unif, 1.0 / N_DIGIT)
            ones_col = big.tile([128, 1], F32, tag="ones_col")
            nc.vector.memset(ones_col, 1.0)
            ones_row = big.tile([1, 128], F16, tag="ones_row")
            nc.vector.memset(ones_row, 1.0)
            ident = big.tile([128, 128], F32, tag="ident")
            make_identity(nc, ident[:])
            eps_t = big.tile([1, 1], F32, tag="eps_t")
            nc.vector.memset(eps_t, EPS)

            # routing logits, both local batches: [i%128, b, k, c, j]  fp32
            bl_t = big.tile([128, BL, NCHUNK, CH, N_DIGIT], F32, tag="bl")

            def trace_loads():
                for b in range(BL):
                    nc.sync.dma_start(out=xk[:, b], in_=xk_d[:, b])
                for b in range(BL):
                    nc.sync.dma_start(out=xt[:, b], in_=xt_d[:, b])
                for gc in range(4):
                    nc.scalar.dma_start(
                        out=g_t[:, gc * 32 : (gc + 1) * 32, :],
                        in_=g_d[:, gc * 32 : (gc + 1) * 32, :],
                    )
                for gc in range(4):
                    nc.scalar.dma_start(
                        out=wt_t[:, gc * 32 : (gc + 1) * 32, :],
                        in_=wt_d[:, gc * 32 : (gc + 1) * 32, :],
                    )

            def trace_body(loads=True, compute=True):
              if loads:
                trace_loads()
              if not compute:
                return
              for t in range(N_ITERS):
                  last = t == N_ITERS - 1

                  # ---- softmax over j (t=0: uniform, skip) ----
                  cb = None
                  if t > 0:
                      mx = sm.tile([128, BL, NCHUNK, CH], F32, tag="mx")
                      eb = sm.tile([128, BL, NCHUNK, CH, N_DIGIT], F32, tag="eb")
                      sb = sm.tile([128, BL, NCHUNK, CH], F32, tag="sum")
                      cb = sm.tile([128, BL, NCHUNK, CH, N_DIGIT], F16, tag="cb")
                      # free-axis reduces are DVE-only; elementwise adds split
                      # DVE (b=0) / GpSimd (b=1) in parallel
                      nc.vector.reduce_max(out=mx[:], in_=bl_t[:], axis=AXX, negate=True)
                      nc.vector.tensor_add(eb[:, 0], bl_t[:, 0], mx[:, 0].to_broadcast(eb[:, 0].shape))
                      nc.gpsimd.tensor_add(eb[:, 1], bl_t[:, 1], mx[:, 1].to_broadcast(eb[:, 1].shape))
                      nc.scalar.activation(eb[:], eb[:], AF.Exp)
                      nc.vector.reduce_sum(out=sb[:], in_=eb[:], axis=AXX)
                      nc.vector.reciprocal(sb[:], sb[:])
                      nc.vector.tensor_mul(cb[:], eb[:], sb[:].to_broadcast(eb.shape))

                  # ---- XC: xcT[q, (c j b)] ----
                  xc_sb = sm.tile([128, CH, N_DIGIT, BL], F16, tag="xc_sb", bufs=3)
                  for c in range(CH):
                      for b in range(BL):
                          xc_ps = ps_xc.tile([128, N_DIGIT], F32, tag="xc_ps")
                          for k in range(NCHUNK):
                              rhs = cb[:, b, k, c, :] if t > 0 else c_unif[:]
                              nc.tensor.matmul(
                                  xc_ps[:],
                                  lhsT=xk[:, b, c, k, :],
                                  rhs=rhs,
                                  start=(k == 0),
                                  stop=(k == NCHUNK - 1),
                              )
                          ve = nc.vector if b == 0 else nc.scalar
                          if b == 0:
                              nc.vector.tensor_copy(xc_sb[:, c, :, b], xc_ps[:])
                          else:
                              nc.scalar.copy(out=xc_sb[:, c, :, b], in_=xc_ps[:])

                  # ---- W-pass: gxT = G @ xc (t<2)  /  sT = W @ xc (t=2) ----
                  wsrc = wt_t if last else g_t
                  gx_ps = ps_gx.tile([128, CH, N_DIGIT, BL], F32, tag="gx")
                  for jc in range(NJC):
                      c, j = divmod(jc, N_DIGIT)
                      nc.tensor.matmul(
                          gx_ps[:, c, j, :],
                          lhsT=wsrc[:, jc, :],
                          rhs=xc_sb[:, c, j, :],
                          start=True,
                          stop=True,
                      )

                  # ---- squash scale kappa; vt = kappa*gx (whole-tile) ----
                  gx_sb = sm.tile([128, CH, N_DIGIT, BL], F16, tag="gx_sb", bufs=3)
                  nc.scalar.copy(out=gx_sb[:], in_=gx_ps[:])
                  xg = sm.tile([128, CH, N_DIGIT, BL], F32, tag="xg")
                  # t<2: |s|^2 = <xc, G xc>;  t=2: |s|^2 = <s, s>
                  if not last:
                      nc.vector.tensor_mul(xg[:], xc_sb[:], gx_sb[:])
                  else:
                      nc.vector.tensor_mul(xg[:], gx_sb[:], gx_sb[:])
                  sq_ps = ps_sq.tile([1, CH * N_DIGIT * BL], F32, tag="sq")
                  nc.tensor.matmul(
                      sq_ps[:],
                      lhsT=ones_col[:],
                      rhs=xg[:].rearrange("p a b c -> p (a b c)"),
                      start=True,
                      stop=True,
                  )
                  # kappa = sq/((1+sq)*sqrt(sq+eps)); sqrt = exp(0.5*ln) so
                  # only the natural_log_exp ACT table set is used.
                  t1 = sm.tile([1, CH * N_DIGIT * BL], F32, tag="t1")
                  t2 = sm.tile([1, CH * N_DIGIT * BL], F32, tag="t2")
                  kap = sm.tile([1, CH * N_DIGIT * BL], F16, tag="kap")
                  nc.scalar.activation(t1[:], sq_ps[:], AF.Ln, bias=eps_t[:])
                  nc.scalar.activation(t1[:], t1[:], AF.Exp, scale=0.5)
                  nc.vector.scalar_tensor_tensor(
                      out=t2[:], in0=sq_ps[:], scalar=1.0,
                      in1=t1[:], op0=AO.add, op1=AO.mult,
                  )
                  nc.vector.reciprocal(t2[:], t2[:])
                  nc.vector.tensor_mul(kap[:], sq_ps[:], t2[:])
                  kb_ps = ps_kb.tile([128, CH, N_DIGIT, BL], F32, tag="kb")
                  nc.tensor.matmul(
                      kb_ps[:].rearrange("p a b c -> p (a b c)"),
                      lhsT=ones_row[:],
                      rhs=kap[:],
                      start=True,
                      stop=True,
                  )
                  # vt (t<2, fp16) or v (t=2, fp32): kappa * gx.  gx_sb (SBUF)
                  # is used since only one tensor input may come from PSUM.
                  if not last:
                      vt = sm.tile([128, CH, N_DIGIT, BL], F16, tag="vt", bufs=3)
                      nc.vector.tensor_mul(vt[:], gx_sb[:], kb_ps[:])
                  else:
                      vt32 = sm.tile([128, CH, N_DIGIT, BL], F32, tag="vt32")
                      nc.vector.tensor_mul(vt32[:], gx_sb[:], kb_ps[:])

                  if not last:
                      # ---- A-pass: a[i,(c j)] = sum_q x[i,q] vt[j,q]; b += a ----
                      # 4 k-chunks share one full PSUM bank so the logits
                      # update is 4 big DVE ops instead of 16 small ones.
                      # (GPSIMD cannot read PSUM, so these stay on DVE/Act.)
                      for b in range(BL):
                          for kh in range(2):
                              a_ps = ps_a.tile([128, 4, CH, N_DIGIT], F32, tag="a")
                              for kk in range(4):
                                  k = kh * 4 + kk
                                  for c in range(CH):
                                      nc.tensor.matmul(
                                          a_ps[:, kk, c, :],
                                          lhsT=xt[:, b, c, k, :],
                                          rhs=vt[:, c, :, b],
                                          start=True,
                                          stop=True,
                                      )
                              dst = bl_t[:, b, kh * 4 : kh * 4 + 4]
                              if t == 0:
                                  if b == 0:
                                      nc.vector.tensor_copy(dst, a_ps[:])
                                  else:
                                      nc.scalar.copy(out=dst, in_=a_ps[:])
                              else:
                                  nc.vector.tensor_add(dst, dst, a_ps[:])
                  else:
                      # ---- output: transpose v [p, (c,j,b)] -> [(c,j,b), p], DMA ----
                      vflat = vt32[:].rearrange("p a b c -> p (a b c)")
                      out_ap = out_d[:].rearrange("b j c p -> c j b p")  # [4,32,2,128]
                      for half in range(2):
                          tr_ps = ps_a.tile([128, 128], F32, tag="a")
                          nc.tensor.transpose(
                              tr_ps[:], vflat[:, half * 128 : (half + 1) * 128], ident[:]
                          )
                          ob = sm.tile([128, 128], F32, tag=f"ob{half}")
                          nc.vector.tensor_copy(ob[:], tr_ps[:])
                          for cl in range(2):
                              nc.sync.dma_start(
                                  out=out_ap[half * 2 + cl],
                                  in_=ob[cl * 64 : (cl + 1) * 64, :],
                              )

            if bench_reps:
                if bench_mode == "nodma":
                    trace_loads()
                with tc.For_i(0, bench_reps, 1):
                    trace_body(loads=(bench_mode != "nodma"),
                               compute=(bench_mode != "dmaonly"))
            else:
                trace_body()
    return nc


def _host_prep(x: np.ndarray, w: np.ndarray):
    """Host-side layout prep shared by all cores (w-derived) and per-core (x)."""
    x = np.ascontiguousarray(x, dtype=np.float32)
    w = np.ascontiguousarray(w, dtype=np.float32)
    # G[j,c,q,r] = sum_p w[j,c,p,q] w[j,c,p,r]
    wf = np.ascontiguousarray(w.transpose(1, 0, 2, 3)).reshape(NJC, D, D)  # jc = c*32+j
    G = np.matmul(wf.transpose(0, 2, 1), wf)                 # [jc, q, r]
    g_h = np.ascontiguousarray(G.transpose(1, 0, 2)).astype(np.float16)    # [q, jc, r]
    wt_h = np.ascontiguousarray(wf.transpose(2, 0, 1)).astype(np.float16)  # [q, jc, p]
    # x[b,i,c,q] with i = k*128 + r  ->  xk [r, b, c, k, q], xt [q, b, c, k, r]
    xr = x.reshape(B, NCHUNK, 128, CH, D)
    xk_h = np.ascontiguousarray(xr.transpose(2, 0, 3, 1, 4)).astype(np.float16)  # [r, b, c, k, q]
    xt_h = np.ascontiguousarray(xr.transpose(4, 0, 3, 1, 2)).astype(np.float16)  # [q, b, c, k, r]
    return xk_h, xt_h, g_h, wt_h


def _run(x: np.ndarray, w: np.ndarray, **spmd_kwargs):
    xk_h, xt_h, g_h, wt_h = _host_prep(x, w)
    in_maps = []
    for core in range(N_CORES):
        in_maps.append(
            {
                "xk": xk_h[:, core * BL : (core + 1) * BL],
                "xt": xt_h[:, core * BL : (core + 1) * BL],
                "g": g_h,
                "wt": wt_h,
            }
        )
    nc = build_nc()
    nc.finalize()
    res = run_bass_kernel_spmd(nc, in_maps, list(range(N_CORES)), **spmd_kwargs)
    out = np.concatenate([res.results[c]["out"] for c in range(N_CORES)], axis=0)
    return out.astype(np.float32), res


def kernel(x: np.ndarray, w: np.ndarray) -> np.ndarray:
    out, _ = _run(x, w)
    return out


# revision 19
# speedup vs baseline: 4.9838x; 1.0210x over previous
"""Trainium2 Bass kernel for CapsNet dynamic routing (nn_Model_16492674417055).

Reference computation:
    u_hat[b,i,j,c,p] = sum_q w[j,c,p,q] x[b,i,c,q]
    3 routing iterations of: c = softmax_j(b); s = sum_i c*u_hat;
    v = squash(s); a = <u_hat, v>; b += a. Output v of last iteration.

Key algebraic factorization (exact in real arithmetic): u_hat never needs to
be materialized (it is 1 GiB).  With xc[b,j,c,:] = sum_i c[b,i,j,c] x[b,i,c,:]:
    s  = W @ xc
    a  = <x_i, W^T v>  and  W^T v = kappa * (W^T W) xc = kappa * G xc,
where kappa is the squash scale, computable from |s|^2 = <xc, G xc>.
So iterations 1..2 need only G = W^T W (host-precomputed), and the final
iteration needs one true W application for the output direction.

Precision: all matmul inputs are fp16 (x, G, wt, softmax weights c, xc, vt);
accumulation is fp32 in PSUM; logits/softmax/squash scalars are fp32.  fp16
(10 mantissa bits) keeps the sharp routing softmax accurate: measured 2.0e-3
relative error on the seed-0 inputs (vs 1.6e-2 for bf16, 3.7e-5 for fp32).
The xc*gx products reach ~6e5 > fp16 max, so xg stays fp32 (fp32 ones-matmul
for |s|^2). fp16 also quarters PE matmul cost vs fp32 and halves DMA.

Sharding: data-parallel over batch B=16 across 8 cores (2 batches/core);
G / wT are replicated (loaded per core); routing state stays core-local.
"""

import numpy as np

import concourse.bass as bass
import concourse.tile as tile
from concourse import bacc
from concourse import mybir
from concourse.alu_op_type import AluOpType as AO
from concourse.bass import MemorySpace
from concourse.bass_utils import run_bass_kernel_spmd
from concourse.masks import make_identity

F32 = mybir.dt.float32
F16 = mybir.dt.float16
AXX = mybir.AxisListType.X
AF = mybir.ActivationFunctionType

N_CORES = 8
B, N_PRE, N_DIGIT, CH, D = 16, 1024, 32, 4, 128
BL = B // N_CORES          # batches per core (2)
NCHUNK = N_PRE // 128      # i-chunks (8)
NJC = N_DIGIT * CH         # 128 (j,c) pairs
EPS = 1e-7
N_ITERS = 3


class _Bacc(bacc.Bacc):
    """Bacc whose ACT-table chooser only sees natural_log_exp_and_others, so
    alternating Exp (softmax) / Ln+Exp (squash sqrt) stay on ONE table set
    (one LoadActFuncSet instead of one per switch)."""

    def insert_act_table_loads(self):
        from concourse.hw_specs import get_activation_tables

        has_activation = any(
            isinstance(i, mybir.InstActivation)
            for b in self.main_func.blocks
            for i in b.instructions
        )
        if not has_activation:
            return
        tables = [
            (n, fns if n == "natural_log_exp_and_others" else set())
            for n, fns in get_activation_tables(self.m.arch).items()
        ]
        bacc._bass_rust.insert_act_table_loads(self, tables)


def build_nc(bench_reps: int = 0, bench_mode: str = "full") -> bass.Bass:
    """bench_reps>0 wraps the whole kernel body (input DMAs included) in a
    For_i loop of that many reps inside one NEFF, for wall-clock timing that
    amortizes the multi-ms axon dispatch floor."""
    nc = _Bacc()

    # Per-core DRAM inputs, host pre-laid-out so every load is a straight
    # [128, N] partition-major copy.  All fp16.
    xk_d = nc.declare_dram_parameter("xk", [128, BL, CH, NCHUNK, 128], F16, isOutput=False)  # [i128, b, c, k, q]
    xt_d = nc.declare_dram_parameter("xt", [128, BL, CH, NCHUNK, 128], F16, isOutput=False)  # [q, b, c, k, i128]
    g_d = nc.declare_dram_parameter("g", [128, NJC, 128], F16, isOutput=False)               # [r, (c j), q]
    wt_d = nc.declare_dram_parameter("wt", [128, NJC, 128], F16, isOutput=False)             # [q, (c j), p]
    out_d = nc.declare_dram_parameter("out", [BL, N_DIGIT, CH, D], F32, isOutput=True)

    with tile.TileContext(nc) as tc:
        with (
            tc.tile_pool(name="big", bufs=1) as big,
            tc.tile_pool(name="sm", bufs=2) as sm,
            tc.tile_pool(name="ps_mix", bufs=2, space=MemorySpace.PSUM) as ps_mix,
            tc.tile_pool(name="ps_gk", bufs=2, space=MemorySpace.PSUM) as ps_gk,
        ):
            # ---- static tiles ----
            xk = big.tile([128, BL, CH, NCHUNK, 128], F16, tag="xk")
            xt = big.tile([128, BL, CH, NCHUNK, 128], F16, tag="xt")
            g_t = big.tile([128, NJC, 128], F16, tag="g")
            wt_t = big.tile([128, NJC, 128], F16, tag="wt")

            c_unif = big.tile([128, N_DIGIT], F16, tag="c_unif")
            nc.vector.memset(c_unif, 1.0 / N_DIGIT)
            ones_col = big.tile([128, 1], F32, tag="ones_col")
            nc.vector.memset(ones_col, 1.0)
            ones_row = big.tile([1, 128], F16, tag="ones_row")
            nc.vector.memset(ones_row, 1.0)
            ident = big.tile([128, 128], F32, tag="ident")
            make_identity(nc, ident[:])
            eps_t = big.tile([1, 1], F32, tag="eps_t")
            nc.vector.memset(eps_t, EPS)

            # routing logits, both local batches: [i%128, b, k, c, j]  fp32
            bl_t = big.tile([128, BL, NCHUNK, CH, N_DIGIT], F32, tag="bl")

            def trace_loads():
                for b in range(BL):
                    nc.sync.dma_start(out=xk[:, b], in_=xk_d[:, b])
                for b in range(BL):
                    nc.sync.dma_start(out=xt[:, b], in_=xt_d[:, b])
                for gc in range(4):
                    nc.scalar.dma_start(
                        out=g_t[:, gc * 32 : (gc + 1) * 32, :],
                        in_=g_d[:, gc * 32 : (gc + 1) * 32, :],
                    )
                for gc in range(4):
                    nc.scalar.dma_start(
                        out=wt_t[:, gc * 32 : (gc + 1) * 32, :],
                        in_=wt_d[:, gc * 32 : (gc + 1) * 32, :],
                    )

            def trace_body(loads=True, compute=True):
              if loads:
                trace_loads()
              if not compute:
                return
              for t in range(N_ITERS):
                  last = t == N_ITERS - 1

                  # ---- softmax over j (t=0: uniform, skip) ----
                  # max-subtract in fp32 (DVE b=0 / GpSimd b=1 in parallel);
                  # the exp output and the rest of the chain are fp16, whose
                  # all-2-byte operands put DVE in 2x-throughput mode.  exp
                  # args are <=0 so fp16 range is safe.
                  cb = None
                  if t > 0:
                      mx = sm.tile([128, BL, NCHUNK, CH], F32, tag="mx")
                      eb = sm.tile([128, BL, NCHUNK, CH, N_DIGIT], F32, tag="eb")
                      e16 = sm.tile([128, BL, NCHUNK, CH, N_DIGIT], F16, tag="e16")
                      sb = sm.tile([128, BL, NCHUNK, CH], F16, tag="sum")
                      cb = sm.tile([128, BL, NCHUNK, CH, N_DIGIT], F16, tag="cb")
                      nc.vector.reduce_max(out=mx[:], in_=bl_t[:], axis=AXX, negate=True)
                      nc.vector.tensor_add(eb[:, 0], bl_t[:, 0], mx[:, 0].to_broadcast(eb[:, 0].shape))
                      nc.gpsimd.tensor_add(eb[:, 1], bl_t[:, 1], mx[:, 1].to_broadcast(eb[:, 1].shape))
                      nc.scalar.activation(e16[:], eb[:], AF.Exp)
                      with nc.allow_low_precision(reason="softmax weights only need ~0.1%; fp16 keeps DVE in 2x mode"):
                          nc.vector.reduce_sum(out=sb[:], in_=e16[:], axis=AXX)
                          nc.vector.reciprocal(sb[:], sb[:])
                      nc.vector.tensor_mul(cb[:], e16[:], sb[:].to_broadcast(e16.shape))

                  # ---- XC: xcT[q, (c j b)] ----
                  xc_sb = sm.tile([128, CH, N_DIGIT, BL], F16, tag="xc_sb", bufs=3)
                  for c in range(CH):
                      for b in range(BL):
                          xc_ps = ps_mix.tile([128, N_DIGIT], F32, tag="xc_ps")
                          for k in range(NCHUNK):
                              rhs = cb[:, b, k, c, :] if t > 0 else c_unif[:]
                              nc.tensor.matmul(
                                  xc_ps[:],
                                  lhsT=xk[:, b, c, k, :],
                                  rhs=rhs,
                                  start=(k == 0),
                                  stop=(k == NCHUNK - 1),
                              )
                          ve = nc.vector if b == 0 else nc.scalar
                          if b == 0:
                              nc.vector.tensor_copy(xc_sb[:, c, :, b], xc_ps[:])
                          else:
                              nc.scalar.copy(out=xc_sb[:, c, :, b], in_=xc_ps[:])

                  # ---- W-pass: gxT = G @ xc (t<2)  /  sT = W @ xc (t=2) ----
                  wsrc = wt_t if last else g_t
                  gx_ps = ps_gk.tile([128, CH, N_DIGIT, BL], F32, tag="gk")
                  for jc in range(NJC):
                      c, j = divmod(jc, N_DIGIT)
                      nc.tensor.matmul(
                          gx_ps[:, c, j, :],
                          lhsT=wsrc[:, jc, :],
                          rhs=xc_sb[:, c, j, :],
                          start=True,
                          stop=True,
                      )

                  # ---- squash scale kappa; vt = kappa*gx (whole-tile) ----
                  gx_sb = sm.tile([128, CH, N_DIGIT, BL], F16, tag="gx_sb", bufs=3)
                  nc.scalar.copy(out=gx_sb[:], in_=gx_ps[:])
                  xg = sm.tile([128, CH, N_DIGIT, BL], F32, tag="xg")
                  # t<2: |s|^2 = <xc, G xc>;  t=2: |s|^2 = <s, s>
                  if not last:
                      nc.vector.tensor_mul(xg[:], xc_sb[:], gx_sb[:])
                  else:
                      nc.vector.tensor_mul(xg[:], gx_sb[:], gx_sb[:])
                  # sq lives in row 0 of the kb bank (saves a PSUM bank); the
                  # later kb matmul overwrites it only after kap is computed.
                  kb_ps = ps_gk.tile([128, CH, N_DIGIT, BL], F32, tag="gk")
                  sq_ps = kb_ps[0:1].rearrange("p a b c -> p (a b c)")
                  nc.tensor.matmul(
                      sq_ps,
                      lhsT=ones_col[:],
                      rhs=xg[:].rearrange("p a b c -> p (a b c)"),
                      start=True,
                      stop=True,
                  )
                  # kappa = sq/((1+sq)*sqrt(sq+eps)); sqrt = exp(0.5*ln) so
                  # only the natural_log_exp ACT table set is used.
                  t1 = sm.tile([1, CH * N_DIGIT * BL], F32, tag="t1")
                  t2 = sm.tile([1, CH * N_DIGIT * BL], F32, tag="t2")
                  kap = sm.tile([1, CH * N_DIGIT * BL], F16, tag="kap")
                  nc.scalar.activation(t1[:], sq_ps, AF.Ln, bias=eps_t[:])
                  nc.scalar.activation(t1[:], t1[:], AF.Exp, scale=0.5)
                  nc.vector.scalar_tensor_tensor(
                      out=t2[:], in0=sq_ps, scalar=1.0,
                      in1=t1[:], op0=AO.add, op1=AO.mult,
                  )
                  nc.vector.reciprocal(t2[:], t2[:])
                  nc.vector.tensor_mul(kap[:], sq_ps, t2[:])
                  nc.tensor.matmul(
                      kb_ps[:].rearrange("p a b c -> p (a b c)"),
                      lhsT=ones_row[:],
                      rhs=kap[:],
                      start=True,
                      stop=True,
                  )
                  # vt (t<2, fp16) or v (t=2, fp32): kappa * gx.  gx_sb (SBUF)
                  # is used since only one tensor input may come from PSUM.
                  if not last:
                      vt = sm.tile([128, CH, N_DIGIT, BL], F16, tag="vt", bufs=3)
                      nc.vector.tensor_mul(vt[:], gx_sb[:], kb_ps[:])
                  else:
                      vt32 = sm.tile([128, CH, N_DIGIT, BL], F32, tag="vt32")
                      nc.vector.tensor_mul(vt32[:], gx_sb[:], kb_ps[:])

                  if not last:
                      # ---- A-pass: a[i,(c j)] = sum_q x[i,q] vt[j,q]; b += a ----
                      # 4 k-chunks share one full PSUM bank so the logits
                      # update is 4 big DVE ops instead of 16 small ones.
                      for b in range(BL):
                          for kh in range(2):
                              a_ps = ps_mix.tile([128, 4, CH, N_DIGIT], F32, tag="a")
                              for kk in range(4):
                                  k = kh * 4 + kk
                                  for c in range(CH):
                                      nc.tensor.matmul(
                                          a_ps[:, kk, c, :],
                                          lhsT=xt[:, b, c, k, :],
                                          rhs=vt[:, c, :, b],
                                          start=True,
                                          stop=True,
                                      )
                              dst = bl_t[:, b, kh * 4 : kh * 4 + 4]
                              if t == 0:
                                  if b == 0:
                                      nc.vector.tensor_copy(dst, a_ps[:])
                                  else:
                                      nc.scalar.copy(out=dst, in_=a_ps[:])
                              else:
                                  nc.vector.tensor_add(dst, dst, a_ps[:])
                  else:
                      # ---- output: transpose v [p, (c,j,b)] -> [(c,j,b), p], DMA ----
                      vflat = vt32[:].rearrange("p a b c -> p (a b c)")
                      out_ap = out_d[:].rearrange("b j c p -> c j b p")  # [4,32,2,128]
                      tr_t = ps_gk.tile([128, CH, N_DIGIT, BL], F32, tag="gk")
                      trv = tr_t[:].rearrange("p a b c -> p (a b c)")
                      for half in range(2):
                          nc.tensor.transpose(
                              trv[:, half * 128 : (half + 1) * 128],
                              vflat[:, half * 128 : (half + 1) * 128], ident[:]
                          )
                      ob = sm.tile([128, 2, 128], F32, tag="ob")
                      nc.vector.tensor_copy(ob[:].rearrange("p a b -> p (a b)"), trv)
                      for half in range(2):
                          for cl in range(2):
                              nc.sync.dma_start(
                                  out=out_ap[half * 2 + cl],
                                  in_=ob[cl * 64 : (cl + 1) * 64, half, :],
                              )

            if bench_reps:
                if bench_mode == "nodma":
                    trace_loads()
                with tc.For_i(0, bench_reps, 1):
                    trace_body(loads=(bench_mode != "nodma"),
                               compute=(bench_mode != "dmaonly"))
            else:
                trace_body()
    return nc


def _host_prep(x: np.ndarray, w: np.ndarray):
    """Host-side layout prep shared by all cores (w-derived) and per-core (x)."""
    x = np.ascontiguousarray(x, dtype=np.float32)
    w = np.ascontiguousarray(w, dtype=np.float32)
    # G[j,c,q,r] = sum_p w[j,c,p,q] w[j,c,p,r]
    wf = np.ascontiguousarray(w.transpose(1, 0, 2, 3)).reshape(NJC, D, D)  # jc = c*32+j
    G = np.matmul(wf.transpose(0, 2, 1), wf)                 # [jc, q, r]
    g_h = np.ascontiguousarray(G.transpose(1, 0, 2)).astype(np.float16)    # [q, jc, r]
    wt_h = np.ascontiguousarray(wf.transpose(2, 0, 1)).astype(np.float16)  # [q, jc, p]
    # x[b,i,c,q] with i = k*128 + r  ->  xk [r, b, c, k, q], xt [q, b, c, k, r]
    xr = x.reshape(B, NCHUNK, 128, CH, D)
    xk_h = np.ascontiguousarray(xr.transpose(2, 0, 3, 1, 4)).astype(np.float16)  # [r, b, c, k, q]
    xt_h = np.ascontiguousarray(xr.transpose(4, 0, 3, 1, 2)).astype(np.float16)  # [q, b, c, k, r]
    return xk_h, xt_h, g_h, wt_h


def _run(x: np.ndarray, w: np.ndarray, **spmd_kwargs):
    xk_h, xt_h, g_h, wt_h = _host_prep(x, w)
    in_maps = []
    for core in range(N_CORES):
        in_maps.append(
            {
                "xk": xk_h[:, core * BL : (core + 1) * BL],
                "xt": xt_h[:, core * BL : (core + 1) * BL],
                "g": g_h,
                "wt": wt_h,
            }
        )
    nc = build_nc()
    nc.finalize()
    res = run_bass_kernel_spmd(nc, in_maps, list(range(N_CORES)), **spmd_kwargs)
    out = np.concatenate([res.results[c]["out"] for c in range(N_CORES)], axis=0)
    return out.astype(np.float32), res


def kernel(x: np.ndarray, w: np.ndarray) -> np.ndarray:
    out, _ = _run(x, w)
    return out
